# revision 1
# baseline (speedup 1.0000x reference)
"""Luong attention energies + softmax on 8 TRN2 NeuronCores.

reference math (per core, batch-sharded):
  energy[b,s] = <hid[b], enc[s,b]> + (hid[b] @ A) . emb[s,b]
  out[b,0,s]  = softmax_s(energy[b,s])

Full shapes: hidden [1,64,512] f32, encoder_outputs [2048,64,512] f32,
embedding [2048,64,3] f32, affect_matrix [512,3] f32 -> out [64,1,2048] f32.

Sharding: batch dim 64 -> 8 cores x 8. No cross-core communication.

Per-core plan (memory-bound: 32 MB encoder shard, ~94 us at 358 GB/s):
  main loop over 16 s-tiles [128p x 8b x 512h] (2 MB DMA each):
    tmp_b = enc_tile[:,b,:] * hid_b      (5 on DVE, 3 on GpSimd)
    Ebuf[:,b,t] = sum_h tmp_b            (6 on ACT accum_out, 2 on DVE reduce)
  epilogue: += affect term, PE identity-transpose, DRAM bounce to [8b, 2048s],
  per-partition softmax (max/exp+accum/recip/mul), contiguous store.
"""

import numpy as np

S, B, H, E = 2048, 64, 512, 3
N_CORES = 8
BS = B // N_CORES      # 8 batches per core
NT = S // 128          # 16 s-tiles of 128 rows
DVE_MULT = (0, 1, 2, 3, 4, 5, 6)   # batches multiplied on DVE
GPS_MULT = (7,)              # batches multiplied on GpSimd
DVE_RED = (3, 4)             # batches reduced on DVE (rest on ACT)

_CACHE = {}


def _build_nc():
    import concourse.bass as bass
    import concourse.tile as tile
    from concourse import bacc, mybir
    from concourse.mybir import AluOpType as alu
    from concourse.mybir import ActivationFunctionType as actf

    f32 = mybir.dt.float32

    nc = bacc.Bacc("TRN2", target_bir_lowering=False, debug=False)
    enc = nc.dram_tensor("enc", [S, BS, H], f32, kind="ExternalInput").ap()
    emb = nc.dram_tensor("emb", [S, BS, E], f32, kind="ExternalInput").ap()
    hid = nc.dram_tensor("hid", [1, BS, H], f32, kind="ExternalInput").ap()
    amat = nc.dram_tensor("amat", [H, E], f32, kind="ExternalInput").ap()
    out = nc.dram_tensor("out", [BS, 1, S], f32, kind="ExternalOutput").ap()
    scr = nc.dram_tensor("scr", [128, 128], f32).ap()   # internal bounce buffer

    with tile.TileContext(nc) as tc:
        with (
            tc.tile_pool(name="persist", bufs=1) as pp,
            tc.tile_pool(name="enc", bufs=5) as encp,
            tc.tile_pool(name="tmp", bufs=3) as tmpp,
            tc.tile_pool(name="junk", bufs=2) as junkp,
            tc.tile_pool(name="psum", bufs=1, space="PSUM") as psp,
        ):
            # ---- hidden broadcast across partitions: [128, BS*H] ----
            hidrow = pp.tile([1, BS * H], f32)
            nc.sync.dma_start(hidrow[:], hid.rearrange("o b h -> o (b h)"))
            hidb = pp.tile([128, BS * H], f32)
            nc.gpsimd.partition_broadcast(hidb[:], hidrow[0:1, :])
            hidb_v = hidb[:].rearrange("p (b h) -> p b h", h=H)

            # ---- identity matrix for the final PE transpose ----
            pidx = pp.tile([128, 1], f32)
            nc.gpsimd.iota(pidx[:], pattern=[[0, 1]], base=0, channel_multiplier=1,
                           allow_small_or_imprecise_dtypes=True)
            colidx = pp.tile([128, 128], f32)
            nc.gpsimd.iota(colidx[:], pattern=[[1, 128]], base=0, channel_multiplier=0,
                           allow_small_or_imprecise_dtypes=True)
            ident = pp.tile([128, 128], f32)
            nc.vector.tensor_scalar(ident[:], colidx[:], pidx[:, 0:1], None, alu.is_equal)

            # ---- main loop: dot partials into Ebuf[p, b, t] ----
            # One batched DVE mult (b0-4), one batched GpSimd mult (b5-7);
            # reduces: one batched DVE reduce (b0-2) + 5 ACT accum reduces.
            ND = len(DVE_MULT)            # 5
            NR = 3                        # batches reduced on DVE
            Ebuf = pp.tile([128, BS * NT], f32)
            Ebuf_v = Ebuf[:].rearrange("p (b t) -> p b t", t=NT)
            for t in range(NT):
                et = encp.tile([128, BS * H], f32, tag="et")
                et_v = et[:].rearrange("p (b h) -> p b h", h=H)
                nc.sync.dma_start(et_v, enc[t * 128:(t + 1) * 128])
                gt = tmpp.tile([128, (BS - ND) * H], f32, tag="gt")
                gt_v = gt[:].rearrange("p (b h) -> p b h", h=H)
                nc.gpsimd.tensor_tensor(gt_v, et_v[:, ND:BS, :],
                                        hidb_v[:, ND:BS, :], alu.mult)
                dt = tmpp.tile([128, ND * H], f32, tag="dt")
                dt_v = dt[:].rearrange("p (b h) -> p b h", h=H)
                nc.vector.tensor_tensor(dt_v, et_v[:, 0:ND, :],
                                        hidb_v[:, 0:ND, :], alu.mult)
                nc.vector.tensor_reduce(
                    Ebuf_v[:, 0:NR, t:t + 1], dt_v[:, 0:NR, :],
                    axis=mybir.AxisListType.X, op=alu.add)
                for b in range(NR, BS):
                    src = dt_v[:, b, :] if b < ND else gt_v[:, b - ND, :]
                    ja = junkp.tile([128, H], f32, tag="jact")
                    nc.scalar.activation(
                        ja[:], src, actf.Copy,
                        accum_out=Ebuf_v[:, b, t:t + 1])

                if t == 7:
                    # ---- hA[b,e] = sum_h hid[b,h] * A[h,e]  (tiny) ----
                    hid8 = pp.tile([BS, H], f32)
                    nc.scalar.dma_start(hid8[:], hid[0])
                    arow = pp.tile([1, H * E], f32)
                    nc.scalar.dma_start(arow[:], amat.rearrange("h e -> (h e)").unsqueeze(0))
                    ab = pp.tile([BS, H * E], f32)
                    nc.gpsimd.partition_broadcast(ab[:], arow[0:1, :])
                    ab_v = ab[:].rearrange("p (h e) -> p h e", e=E)
                    hA = pp.tile([BS, E], f32)
                    for e in range(E):
                        j8 = junkp.tile([BS, H], f32, tag="j8")
                        nc.vector.tensor_tensor(j8[:], hid8[:], ab_v[:, :, e], alu.mult)
                        nc.vector.tensor_reduce(hA[:, e:e + 1], j8[:],
                                                axis=mybir.AxisListType.X, op=alu.add)
                    # flatten hA [BS,E] partitions -> single row [1, BS*E], then bcast
                    harow = pp.tile([1, BS * E], f32)
                    nc.scalar.dma_start(harow[0:1].rearrange("o (b e) -> o b e", e=E), hA[:])
                    hab = pp.tile([128, BS * E], f32)
                    nc.gpsimd.partition_broadcast(hab[:], harow[0:1, :])

                    # ---- aff[p, t, b] = sum_e emb[t*128+p, b, e] * hA[b, e] ----
                    emba = pp.tile([128, NT * BS * E], f32)
                    emba_v = emba[:].rearrange("p (t b e) -> p t b e", b=BS, e=E)
                    nc.scalar.dma_start(emba_v, emb.rearrange("(t p) b e -> p t b e", p=128))
                    afftmp = pp.tile([128, NT * BS * E], f32)
                    hab_bv = (hab[:].rearrange("p (b e) -> p b e", e=E)
                              .unsqueeze(1).broadcast_to([128, NT, BS, E]))
                    nc.vector.tensor_tensor(
                        afftmp[:].rearrange("p (t b e) -> p t b e", b=BS, e=E),
                        emba_v, hab_bv, alu.mult)
                    aff = pp.tile([128, NT * BS], f32)
                    aff_v = aff[:].rearrange("p (t b) -> p t b", b=BS)
                    nc.vector.tensor_reduce(
                        aff_v, afftmp[:].rearrange("p (t b e) -> p t b e", b=BS, e=E),
                        axis=mybir.AxisListType.X, op=alu.add)


            # add the affect term for all batches (one strided op)
            nc.vector.tensor_tensor(
                Ebuf_v, Ebuf_v, aff_v.transpose([0, 2, 1]), alu.add)

            # ---- transpose [p,(b,t)] -> [(b,t),p], bounce via DRAM to [b, s] ----
            pt = psp.tile([128, 128], f32)
            nc.tensor.transpose(pt[:], Ebuf[:], ident[:])
            T1 = pp.tile([128, 128], f32)
            nc.scalar.copy(T1[:], pt[:])
            nc.sync.dma_start(scr, T1[:])
            Eb2 = pp.tile([BS, S], f32)
            nc.sync.dma_start(Eb2[:], scr.rearrange("(b t) p -> b (t p)", b=BS))

            # ---- softmax over s, one op each ----
            mx = pp.tile([BS, 1], f32)
            nc.vector.tensor_reduce(mx[:], Eb2[:], axis=mybir.AxisListType.X, op=alu.max)
            ngm = pp.tile([BS, 1], f32)
            nc.vector.tensor_scalar_mul(ngm[:], mx[:], -1.0)
            Pb = pp.tile([BS, S], f32)
            sm = pp.tile([BS, 1], f32)
            nc.scalar.activation(Pb[:], Eb2[:], actf.Exp,
                                 bias=ngm[:, 0:1], scale=1.0, accum_out=sm[:, 0:1])
            rec = pp.tile([BS, 1], f32)
            nc.vector.reciprocal(rec[:], sm[:])
            nc.vector.tensor_scalar(Pb[:], Pb[:], rec[:, 0:1], None, alu.mult)
            nc.sync.dma_start(out.rearrange("b o s -> b (o s)"), Pb[:])

    nc.compile()
    return nc


def _get_nc():
    if "nc" not in _CACHE:
        _CACHE["nc"] = _build_nc()
    return _CACHE["nc"]


def kernel(hidden, encoder_outputs, embedding, affect_matrix):
    from concourse.bass_utils import run_bass_kernel_spmd

    nc = _get_nc()
    hidden = np.asarray(hidden, dtype=np.float32)
    encoder_outputs = np.asarray(encoder_outputs, dtype=np.float32)
    embedding = np.asarray(embedding, dtype=np.float32)
    affect_matrix = np.asarray(affect_matrix, dtype=np.float32)

    in_maps = []
    for c in range(N_CORES):
        sl = slice(c * BS, (c + 1) * BS)
        in_maps.append({
            "enc": np.ascontiguousarray(encoder_outputs[:, sl, :]),
            "emb": np.ascontiguousarray(embedding[:, sl, :]),
            "hid": np.ascontiguousarray(hidden[:, sl, :]),
            "amat": affect_matrix,
        })
    res = run_bass_kernel_spmd(nc, in_maps, list(range(N_CORES)))
    return np.concatenate([res.results[c]["out"] for c in range(N_CORES)], axis=0)

